# revision 1
# baseline (speedup 1.0000x reference)
"""Trainium2 Bass kernel for the CGC multi-task MoE routing problem.

Full-input contract: kernel(**inputs) takes the unsharded numpy inputs and
returns the full [T+1, B, E] float32 output.

Strategy: pure data-parallel over batch across 8 NeuronCores (weights
replicated, no collectives). Per core (B_loc = 1024):
  - host pre-transposes activations to feature-major xT [D, B_loc] bf16
  - 16 experts (12 task-specific + 4 shared), each a 2-layer ReLU MLP,
    computed feature-major on TensorE in bf16:
        hT[H,B] = relu(W1.T @ xT + b1);  sT[E,B] = relu(W2.T @ hT + b2)
  - gate logits computed in [B, n_exp] orientation (lhsT = xT chunks),
    softmax along the free dim (exp on ScalarE with accum_out row-sum)
  - each expert output tile is PE-transposed to [B, E] and accumulated
    into per-task + shared-pool f32 accumulators with one fused
    scalar_tensor_tensor (acc = sT * gate + acc) per contribution
"""

import numpy as np
import ml_dtypes

import concourse.bass as bass
import concourse.mybir as mybir
from concourse.tile import TileContext
from concourse.bass_utils import run_bass_kernel_spmd

BF16 = ml_dtypes.bfloat16

# Problem shapes (hardcoded per spec)
T, B, D, H, E = 3, 8192, 1024, 512, 256
S, NS = 4, 4
NCORES = 8
BL = B // NCORES          # per-core batch rows (1024)
NBT = BL // 128           # b-tiles of 128 per core (8)
KD = D // 128             # contraction chunks for layer 1 (8)
KH = H // 128             # contraction chunks for layer 2 (4)
NEXP = T * S + NS         # 16 experts total
BN = 512                  # matmul moving free-dim chunk (1 PSUM bank of f32)
NBN = BL // BN            # 2

TRACE = False             # test harness sets kernel.TRACE = True for profiling
LAST_EXEC_NS = None

_CACHE = {}

# this walrus build rejects instructions carrying more than one semaphore wait
# condition ("Too many sync wait commands" in CoreV3 setupSyncWait; observed on
# Drain with 2+ and TensorTensor with 2), but Tile's sem-assigner and tail
# drain emit up to ~11 on one instruction
DRAIN_KEEP = 1
OTHER_KEEP = 1


def _split_excess_waits(nc):
    """Move overflow sem-waits onto same-engine NOPs inserted just before the
    overloaded instruction. Waiting earlier on the same engine preserves the
    ordering guarantee the wait provides."""
    n_split = 0
    for f in nc.m.functions:
        for bb in f.blocks:
            insts = bb.instructions
            need = False
            for i in insts:
                si = i.sync_info
                if si and si.on_wait and len(si.on_wait) > (
                    DRAIN_KEEP if isinstance(i, mybir.InstDrain) else OTHER_KEEP
                ):
                    need = True
                    break
            if not need:
                continue
            new_insts = []
            for inst in insts:
                si = inst.sync_info
                waits = list(si.on_wait) if si and si.on_wait else []
                keep = DRAIN_KEEP if isinstance(inst, mybir.InstDrain) else OTHER_KEEP
                if len(waits) > keep:
                    overflow = waits[: len(waits) - keep]
                    si.on_wait = waits[len(waits) - keep :]
                    for k, w in enumerate(overflow):
                        nop = mybir.InstNoOp(
                            name=f"{inst.name}-wsplit{k}", ins=[], outs=[]
                        )
                        nop.engine = inst.engine
                        nop.sync_info = mybir.SyncInfo(on_wait=[w], on_update=[])
                        new_insts.append(nop)
                        n_split += 1
                new_insts.append(inst)
            bb.instructions = new_insts
    return n_split


def _check_read_before_write(nc):
    """Emission-order lint: an on-chip tile read before any write means Tile
    will schedule the consumer against uninitialized memory (the bug class
    behind two earlier gate_sb/bg_sb ordering regressions)."""
    import sys

    written = set()
    flagged = set()
    for f in nc.m.functions:
        for bb in f.blocks:
            for inst in bb.instructions:
                for arg in inst.ins:
                    t = getattr(getattr(arg, "bass_ap", None), "tensor", None)
                    name = getattr(t, "name", None)
                    if name and name not in written and name not in flagged:
                        space = getattr(t, "space", None)
                        if str(space) in ("MemorySpace.SBUF", "MemorySpace.PSUM"):
                            flagged.add(name)
                            print(
                                f"WARNING: {inst.name} reads {name} before any "
                                f"write (emission order)",
                                file=sys.stderr,
                            )
                for arg in inst.outs:
                    t = getattr(getattr(arg, "bass_ap", None), "tensor", None)
                    name = getattr(t, "name", None)
                    if name:
                        written.add(name)


def _build_program(split_waits=True):
    f32 = mybir.dt.float32
    bf16 = mybir.dt.bfloat16
    relu = mybir.ActivationFunctionType.Relu
    expf = mybir.ActivationFunctionType.Exp
    mult = mybir.AluOpType.mult
    add = mybir.AluOpType.add

    nc = bass.Bass()
    xT = nc.dram_tensor("xT", [4, D, BL], bf16, kind="ExternalInput")
    w1 = nc.dram_tensor("w1", [NEXP, D, H], bf16, kind="ExternalInput")
    w2 = nc.dram_tensor("w2", [NEXP, H, E], bf16, kind="ExternalInput")
    b1 = nc.dram_tensor("b1", [NEXP, 128, KH], f32, kind="ExternalInput")
    b2 = nc.dram_tensor("b2", [NEXP, 128, E // 128], f32, kind="ExternalInput")
    wg = nc.dram_tensor("wg", [128, 4 * KD * 16], bf16, kind="ExternalInput")
    bg = nc.dram_tensor("bg", [128, 4 * 16], f32, kind="ExternalInput")
    ident = nc.dram_tensor("ident", [128, 128], bf16, kind="ExternalInput")
    out = nc.dram_tensor("out", [4, BL, E], f32, kind="ExternalOutput")

    with TileContext(nc) as tc:
        with (
            tc.tile_pool(name="const", bufs=1) as constp,
            tc.tile_pool(name="xp", bufs=1) as xp,
            tc.tile_pool(name="accp", bufs=1) as accp,
            tc.tile_pool(name="w1p", bufs=3) as w1p,
            tc.tile_pool(name="w2p", bufs=3) as w2p,
            tc.tile_pool(name="bp", bufs=4) as bp,
            tc.tile_pool(name="hp", bufs=3) as hp,
            tc.tile_pool(name="sp", bufs=3) as sp,
            tc.tile_pool(name="gp", bufs=4) as gp,
            tc.tile_pool(name="shp", bufs=8) as shp,
            tc.tile_pool(name="psh", bufs=3, space="PSUM") as psh_pool,
            tc.tile_pool(name="pss", bufs=2, space="PSUM") as pss_pool,
            tc.tile_pool(name="pst", bufs=2, space="PSUM") as pst_pool,
            tc.tile_pool(name="psg", bufs=1, space="PSUM") as psg_pool,
        ):
            wg_sb = constp.tile([128, 4 * KD * 16], bf16)
            bg_sb = constp.tile([128, 4 * 16], f32)
            id_sb = constp.tile([128, 128], bf16)

            # x tiles: chunked DMAs so consumers start as chunks land.
            # shared input (src 3) first: the shared gate is computed first.
            xt_sb = [
                xp.tile([128, KD * BL], bf16, name=f"xt{src}") for src in range(4)
            ]

            def load_xt(src):
                for c in range(KD):
                    nc.sync.dma_start(
                        out=xt_sb[src][:, c * BL : (c + 1) * BL],
                        in_=xT[src][c * 128 : (c + 1) * 128, :],
                    )

            acc = [accp.tile([128, NBT * E], f32, name=f"acc{t}") for t in range(4)]

            def next_pst():
                return pst_pool.tile([128, E], bf16, tag="pst", name="ps_t")
            gate_sb = [
                constp.tile([128, NBT * 16], f32, name=f"gate{s}") for s in range(4)
            ]
            written = set()  # (acc_idx, bt) already initialized

            def emit_gates(src):
                wexp = 8 if src < 3 else 16
                # one PSUM bank holds all NBT b-tiles of this gate set;
                # c-inner order so each xt chunk is consumed as it arrives
                # all NBT b-tiles form ONE psum accumulation group: start=True
                # lazily zeroes the whole 2KB zero-region, so each slice's
                # first write initializes it and later writes accumulate.
                # Interleaved per-slice groups would wipe siblings' partials.
                psg = psg_pool.tile([128, NBT * 16], f32)
                for c in range(KD):
                    for bt in range(NBT):
                        nc.tensor.matmul(
                            psg[:, bt * 16 : bt * 16 + 16],
                            lhsT=xt_sb[src][:, c * BL + bt * 128 : c * BL + bt * 128 + 128],
                            rhs=wg_sb[:, (src * KD + c) * 16 : (src * KD + c) * 16 + 16],
                            start=(c == 0 and bt == 0),
                            stop=(c == KD - 1 and bt == NBT - 1),
                        )
                # single fast eviction so the PSUM bank frees immediately;
                # softmax then runs off SBUF without stalling the next gate set
                logits = gp.tile([128, NBT * 16], f32, tag="logits")
                nc.scalar.copy(logits, psg)
                for bt in range(NBT):
                    logit = gp.tile([128, 16], f32, tag="logit")
                    nc.vector.tensor_add(
                        logit[:, :wexp],
                        logits[:, bt * 16 : bt * 16 + wexp],
                        bg_sb[:, src * 16 : src * 16 + wexp],
                    )
                    g_ap = gate_sb[src][:, bt * 16 : bt * 16 + wexp]
                    ssum = gp.tile([128, 1], f32, tag="ssum")
                    nc.scalar.activation(g_ap, logit[:, :wexp], expf, accum_out=ssum)
                    rsum = gp.tile([128, 1], f32, tag="rsum")
                    nc.vector.reciprocal(rsum, ssum)
                    nc.vector.tensor_scalar_mul(g_ap, g_ap, rsum)

            # SBUF staging for shared experts computed before the task gates
            # exist: their transposed [B,E] tiles wait here until the gates
            # are ready and the deferred combine runs
            stage = {}

            def load_w1(e):
                w1_sb = w1p.tile([128, KD * H], bf16)
                nc.sync.dma_start(
                    out=w1_sb.rearrange("p (c h) -> p c h", c=KD),
                    in_=w1[e].rearrange("(c p) h -> p c h", p=128),
                )
                return w1_sb

            DEFAULT_BN = [(0, BN), (BN, BN)]

            def emit_expert(e, src, finalize, defer=False, extra_per_bt=None,
                            w1_pre=None, h_pre=None, bn_list=DEFAULT_BN):
                w1_sb = w1_pre if w1_pre is not None else load_w1(e)
                w2_sb = w2p.tile([128, KH * E], bf16)
                nc.sync.dma_start(
                    out=w2_sb.rearrange("p (c f) -> p c f", c=KH),
                    in_=w2[e].rearrange("(c p) f -> p c f", p=128),
                )
                b1_sb = bp.tile([128, KH], f32, tag="b1")
                nc.sync.dma_start(out=b1_sb, in_=b1[e])
                b2_sb = bp.tile([128, E // 128], f32, tag="b2")
                nc.sync.dma_start(out=b2_sb, in_=b2[e])

                for off, W in bn_list:
                    if h_pre is not None and off == 0:
                        h_sb = h_pre
                    else:
                        h_sb = hp.tile([128, KH * W], bf16, name="h_sb", tag="h_sb")
                        for hc in range(KH):
                            ps_h = psh_pool.tile([128, W], f32, name="ps_h", tag="ps_h")
                            for c in range(KD):
                                nc.tensor.matmul(
                                    ps_h,
                                    lhsT=w1_sb[:, c * H + hc * 128 : c * H + hc * 128 + 128],
                                    rhs=xt_sb[src][:, c * BL + off : c * BL + off + W],
                                    start=(c == 0),
                                    stop=(c == KD - 1),
                                )
                            nc.scalar.activation(
                                h_sb[:, hc * W : (hc + 1) * W],
                                ps_h,
                                relu,
                                bias=b1_sb[:, hc : hc + 1],
                            )
                    s_sb = sp.tile([128, 2 * W], bf16, name="s_sb", tag="s_sb")
                    for ec in range(2):
                        ps_s = pss_pool.tile([128, W], f32, name="ps_s", tag="ps_s")
                        for hc in range(KH):
                            nc.tensor.matmul(
                                ps_s,
                                lhsT=w2_sb[:, hc * E + ec * 128 : hc * E + ec * 128 + 128],
                                rhs=h_sb[:, hc * W : (hc + 1) * W],
                                start=(hc == 0),
                                stop=(hc == KH - 1),
                            )
                        nc.scalar.activation(
                            s_sb[:, ec * W : (ec + 1) * W],
                            ps_s,
                            relu,
                            bias=b2_sb[:, ec : ec + 1],
                        )
                    for j in range(W // 128):
                        bt = off // 128 + j
                        ps_t = next_pst()
                        for ec in range(2):
                            nc.tensor.transpose(
                                ps_t[:, ec * 128 : (ec + 1) * 128],
                                s_sb[:, ec * W + j * 128 : ec * W + j * 128 + 128],
                                id_sb,
                            )

                        if defer:
                            st = shp.tile([128, E], bf16, tag=f"st{e}")
                            nc.scalar.copy(st, ps_t)
                            stage[(e, bt)] = st
                        else:
                            emit_contribs(e, bt, ps_t)
                        if extra_per_bt is not None:
                            extra_per_bt(bt)

                        # flush finished accumulator chunks to DRAM as soon as
                        # their last contribution lands (batched per chunk to
                        # amortize DMA descriptor latency)
                        if j == W // 128 - 1:
                            nb = W // 128
                            b0 = off // 128
                            for t in finalize:
                                nc.sync.dma_start(
                                    out=out[t][off : off + W, :].rearrange(
                                        "(b p) f -> p b f", p=128
                                    ),
                                    in_=acc[t][
                                        :, b0 * E : (b0 + nb) * E
                                    ].rearrange("p (b f) -> p b f", b=nb),
                                )

            def contribs_of(e):
                if e < T * S:
                    t, s = divmod(e, S)
                    return [(t, s), (3, t * S + s)]
                jsh = e - T * S
                return [(t, S + jsh) for t in range(T)] + [(3, T * S + jsh)]

            def emit_contribs(e, bt, src_tile):
                for gset, col in contribs_of(e):
                    g = gate_sb[gset][:, bt * 16 + col : bt * 16 + col + 1]
                    a = acc[gset][:, bt * E : (bt + 1) * E]
                    if (gset, bt) not in written:
                        written.add((gset, bt))
                        nc.vector.tensor_scalar_mul(a, src_tile, g)
                    else:
                        nc.vector.scalar_tensor_tensor(
                            a, src_tile, g, a, op0=mult, op1=add
                        )

            # Emission = engine program order. Shared experts first: they need
            # no gates at compute time (combine deferred via SBUF staging), so
            # PE ramps while only xt3 + their weights are in flight. Task
            # gates follow, then spec experts with the deferred shared-pool
            # contributions interleaved per b-tile. Tail = e11 (2 contribs).
            # acc0 ends with e3's hook (deferred e15), acc1 with e7,
            # acc2 / acc3 with e11
            finalize_at = {3: [0], 7: [1], 11: [2, 3]}

            # Prologue: xt3 and w1[12] chunk DMAs interleaved; e12's first-half
            # layer-1 runs c-outer across 3 PSUM banks so PE consumes each
            # (xt3, w1) chunk pair as it lands instead of idling on the load.
            w1_12 = w1p.tile([128, KD * H], bf16, name="w1_12")
            for c in range(KD):
                nc.sync.dma_start(
                    out=xt_sb[3][:, c * BL : (c + 1) * BL],
                    in_=xT[3][c * 128 : (c + 1) * 128, :],
                )
                nc.sync.dma_start(
                    out=w1_12[:, c * H : (c + 1) * H],
                    in_=w1[12][c * 128 : (c + 1) * 128, :],
                )
                if c == 1:
                    # mid-stream so it lands well before the gate(3) matmuls
                    # start, without delaying the first chunk pair
                    nc.sync.dma_start(out=wg_sb, in_=wg[:, :])
            b1_12 = bp.tile([128, KH], f32, tag="b1", name="b1_12")
            nc.sync.dma_start(out=b1_12, in_=b1[12])
            nc.sync.dma_start(out=bg_sb, in_=bg[:, :])
            nc.sync.dma_start(out=id_sb, in_=ident[:, :])

            h12 = hp.tile([128, KH * BN], bf16, name="h12")
            ph = [
                psh_pool.tile([128, BN], f32, name=f"ph{hc}", tag="ps_h")
                for hc in range(3)
            ]
            for c in range(KD):
                for hc in range(3):
                    nc.tensor.matmul(
                        ph[hc],
                        lhsT=w1_12[:, c * H + hc * 128 : c * H + hc * 128 + 128],
                        rhs=xt_sb[3][:, c * BL : c * BL + BN],
                        start=(c == 0),
                        stop=(c == KD - 1),
                    )
            for hc in range(3):
                nc.scalar.activation(
                    h12[:, hc * BN : (hc + 1) * BN], ph[hc], relu,
                    bias=b1_12[:, hc : hc + 1],
                )
            ph3 = psh_pool.tile([128, BN], f32, name="ph3", tag="ps_h")
            for c in range(KD):
                nc.tensor.matmul(
                    ph3,
                    lhsT=w1_12[:, c * H + 3 * 128 : c * H + 3 * 128 + 128],
                    rhs=xt_sb[3][:, c * BL : c * BL + BN],
                    start=(c == 0),
                    stop=(c == KD - 1),
                )
            nc.scalar.activation(
                h12[:, 3 * BN : 4 * BN], ph3, relu, bias=b1_12[:, 3:4]
            )
            emit_gates(3)
            emit_expert(12, 3, [], defer=True, w1_pre=w1_12, h_pre=h12)
            load_xt(0)
            emit_gates(0)
            emit_expert(13, 3, [], defer=True)
            emit_expert(14, 3, [], defer=True)
            load_xt(1)
            emit_gates(1)
            emit_expert(15, 3, [], defer=True)
            load_xt(2)
            emit_gates(2)

            def make_hook(shared_e):
                def hook(bt):
                    emit_contribs(shared_e, bt, stage[(shared_e, bt)])
                return hook

            for e in [0, 1, 2, 3]:
                emit_expert(e, 0, finalize_at.get(e, []),
                            extra_per_bt=make_hook(12 + e))
            for e in [4, 5, 6, 7]:
                emit_expert(e, 1, finalize_at.get(e, []))
            for e in [8, 9, 10]:
                emit_expert(e, 2, finalize_at.get(e, []))
            # last expert runs progressively finer column chunks so the final
            # combine + accumulator flush pipeline covers only 128 rows
            emit_expert(11, 2, finalize_at[11],
                        bn_list=[(0, 512), (512, 256), (768, 128), (896, 128)])

    _check_read_before_write(nc)
    if split_waits:
        _split_excess_waits(nc)
    return nc


def _prep_shared(W_spec1, b_spec1, W_spec2, b_spec2, W_sh1, b_sh1, W_sh2, b_sh2,
                 W_gate, b_gate, W_gate_sh, b_gate_sh):
    """Host-side prep of the replicated (per-core-identical) tensors."""
    w1 = np.ascontiguousarray(
        np.concatenate([W_spec1, W_sh1], axis=0).astype(BF16)
    )
    w2 = np.ascontiguousarray(
        np.concatenate([W_spec2, W_sh2], axis=0).astype(BF16)
    )
    b1 = np.ascontiguousarray(
        np.concatenate([b_spec1, b_sh1], axis=0)
        .astype(np.float32)
        .reshape(NEXP, KH, 128)
        .transpose(0, 2, 1)
    )
    b2 = np.ascontiguousarray(
        np.concatenate([b_spec2, b_sh2], axis=0)
        .astype(np.float32)
        .reshape(NEXP, E // 128, 128)
        .transpose(0, 2, 1)
    )
    # gate weights: [4 sets, D, 16] (task gates padded 8 -> 16 with zeros)
    wg_full = np.zeros((4, D, 16), np.float32)
    wg_full[:T, :, : S + NS] = W_gate
    wg_full[3] = W_gate_sh
    # device layout [128, (set, chunk, e)]
    wg = np.ascontiguousarray(
        wg_full.reshape(4, KD, 128, 16).transpose(2, 0, 1, 3).reshape(128, 4 * KD * 16)
    ).astype(BF16)
    bg_full = np.zeros((4, 16), np.float32)
    bg_full[:T, : S + NS] = b_gate
    bg_full[3] = b_gate_sh
    bg = np.ascontiguousarray(
        np.broadcast_to(bg_full[None], (128, 4, 16)).reshape(128, 64)
    )
    ident = np.eye(128, dtype=BF16)
    return dict(w1=w1, w2=w2, b1=b1, b2=b2, wg=wg, bg=bg, ident=ident)


def kernel(x_tasks, x_shared, W_spec1, b_spec1, W_spec2, b_spec2,
           W_sh1, b_sh1, W_sh2, b_sh2, W_gate, b_gate, W_gate_sh, b_gate_sh):
    global LAST_EXEC_NS
    if "nc" not in _CACHE:
        _CACHE["nc"] = _build_program()
    nc = _CACHE["nc"]

    shared = _prep_shared(W_spec1, b_spec1, W_spec2, b_spec2, W_sh1, b_sh1,
                          W_sh2, b_sh2, W_gate, b_gate, W_gate_sh, b_gate_sh)

    x_tasks = np.asarray(x_tasks, np.float32)
    x_shared = np.asarray(x_shared, np.float32)

    in_maps = []
    for i in range(NCORES):
        sl = slice(i * BL, (i + 1) * BL)
        xt = np.empty((4, D, BL), BF16)
        for t in range(T):
            xt[t] = x_tasks[t, sl, :].T.astype(BF16)
        xt[3] = x_shared[sl, :].T.astype(BF16)
        m = {"xT": xt}
        m.update(shared)
        in_maps.append(m)

    res = run_bass_kernel_spmd(nc, in_maps, core_ids=list(range(NCORES)), trace=TRACE)
    LAST_EXEC_NS = res.exec_time_ns

    full = np.empty((T + 1, B, E), np.float32)
    for i in range(NCORES):
        full[:, i * BL : (i + 1) * BL, :] = res.results[i]["out"]
    return full



# revision 5
# speedup vs baseline: 1.5382x; 1.5382x over previous
"""Trainium2 Bass kernel for the CGC multi-task MoE routing problem.

Full-input contract: kernel(**inputs) takes the unsharded numpy inputs and
returns the full [T+1, B, E] float32 output.

Strategy: pure data-parallel over batch across 8 NeuronCores (weights
replicated, no collectives). Per core (B_loc = 1024):
  - layer 1 runs on TensorE in fp8e4m3 DoubleRow mode (2 contraction
    chunks per instruction at 0.5 cycles/row): x is host-split into an
    (hi, lo) e4m3 pair so its quantization error is compensated; W1 is
    naive e4m3 scaled by 64 (the single uncompensated noise source,
    ~1.6e-2 output rel err, inside the 2e-2 budget)
  - layer 2 runs in bf16 with the output directly in [B, E] orientation
    (lhsT = hT chunks), which removes all PE transposes of the baseline
  - gate logits are computed from the same fp8 x pair against an
    (hi, lo) e4m3 gate-weight pair (3-term compensation); the 1/64
    weight scale is folded into the softmax's bias-add
  - combine: acc = s * gate + acc fused on VectorE; layer-2 relu on
    GpSimd, layer-1 relu+bias on ScalarE to balance engines
"""

import numpy as np
import ml_dtypes

import concourse.bass as bass
import concourse.mybir as mybir
from concourse.tile import TileContext
from concourse.bass_utils import run_bass_kernel_spmd

BF16 = ml_dtypes.bfloat16
E4M3 = ml_dtypes.float8_e4m3

# Problem shapes (hardcoded per spec)
T, B, D, H, E = 3, 8192, 1024, 512, 256
S, NS = 4, 4
NCORES = 8
BL = B // NCORES          # per-core batch rows (1024)
NBT = BL // 128           # b-tiles of 128 per core (8)
KD = D // 128             # contraction chunks for layer 1 (8)
KH = H // 128             # contraction chunks for layer 2 (4)
NEXP = T * S + NS         # 16 experts total
BN = 512                  # layer-1 moving free-dim chunk (1 PSUM bank of f32)
WSCALE = 64.0             # fp8 weight scale (W1, wg); undone via W2/softmax

TRACE = False             # test harness sets kernel.TRACE = True for profiling
LAST_EXEC_NS = None

_CACHE = {}

# this walrus build rejects instructions carrying more than one semaphore wait
# condition ("Too many sync wait commands" in CoreV3 setupSyncWait; observed on
# Drain with 2+ and TensorTensor with 2), but Tile's sem-assigner and tail
# drain emit up to ~11 on one instruction
DRAIN_KEEP = 1
OTHER_KEEP = 1


def _split_excess_waits(nc):
    """Move overflow sem-waits onto same-engine NOPs inserted just before the
    overloaded instruction. Waiting earlier on the same engine preserves the
    ordering guarantee the wait provides."""
    n_split = 0
    for f in nc.m.functions:
        for bb in f.blocks:
            insts = bb.instructions
            need = False
            for i in insts:
                si = i.sync_info
                if si and si.on_wait and len(si.on_wait) > (
                    DRAIN_KEEP if isinstance(i, mybir.InstDrain) else OTHER_KEEP
                ):
                    need = True
                    break
            if not need:
                continue
            new_insts = []
            for inst in insts:
                si = inst.sync_info
                waits = list(si.on_wait) if si and si.on_wait else []
                keep = DRAIN_KEEP if isinstance(inst, mybir.InstDrain) else OTHER_KEEP
                if len(waits) > keep:
                    overflow = waits[: len(waits) - keep]
                    si.on_wait = waits[len(waits) - keep :]
                    for k, w in enumerate(overflow):
                        nop = mybir.InstNoOp(
                            name=f"{inst.name}-wsplit{k}", ins=[], outs=[]
                        )
                        nop.engine = inst.engine
                        nop.sync_info = mybir.SyncInfo(on_wait=[w], on_update=[])
                        new_insts.append(nop)
                        n_split += 1
                new_insts.append(inst)
            bb.instructions = new_insts
    return n_split


def _check_read_before_write(nc):
    """Emission-order lint: an on-chip tile read before any write means Tile
    will schedule the consumer against uninitialized memory."""
    import sys

    written = set()
    flagged = set()
    for f in nc.m.functions:
        for bb in f.blocks:
            for inst in bb.instructions:
                for arg in inst.ins:
                    t = getattr(getattr(arg, "bass_ap", None), "tensor", None)
                    name = getattr(t, "name", None)
                    if name and name not in written and name not in flagged:
                        space = getattr(t, "space", None)
                        if str(space) in ("MemorySpace.SBUF", "MemorySpace.PSUM"):
                            flagged.add(name)
                            print(
                                f"WARNING: {inst.name} reads {name} before any "
                                f"write (emission order)",
                                file=sys.stderr,
                            )
                for arg in inst.outs:
                    t = getattr(getattr(arg, "bass_ap", None), "tensor", None)
                    name = getattr(t, "name", None)
                    if name:
                        written.add(name)
    return flagged


def _build_program(split_waits=True, use_b2=False):
    f32 = mybir.dt.float32
    bf16 = mybir.dt.bfloat16
    fp8 = mybir.dt.float8e4
    relu = mybir.ActivationFunctionType.Relu
    expf = mybir.ActivationFunctionType.Exp
    mult = mybir.AluOpType.mult
    add = mybir.AluOpType.add
    maxop = mybir.AluOpType.max
    DR = mybir.MatmulPerfMode.DoubleRow

    nc = bass.Bass()
    # x: [src, hi/lo, d, b] feature-major fp8 pair
    xT = nc.dram_tensor("xT", [4, 2, D, BL], fp8, kind="ExternalInput")
    w1 = nc.dram_tensor("w1", [NEXP, D, H], fp8, kind="ExternalInput")
    w2 = nc.dram_tensor("w2", [NEXP, H, E], bf16, kind="ExternalInput")
    b1 = nc.dram_tensor("b1", [NEXP, 128, KH], f32, kind="ExternalInput")
    # wg: [hi/lo][128, (set, c, 16)] fp8 pair
    wg = nc.dram_tensor("wg", [2, 128, 4 * KD * 16], fp8, kind="ExternalInput")
    bg = nc.dram_tensor("bg", [128, 4 * 16], f32, kind="ExternalInput")
    if use_b2:
        b2bc = nc.dram_tensor("b2bc", [NEXP, 128, E], f32, kind="ExternalInput")
    out = nc.dram_tensor("out", [4, BL, E], f32, kind="ExternalOutput")

    with TileContext(nc) as tc:
        with (
            tc.tile_pool(name="const", bufs=1) as constp,
            tc.tile_pool(name="xp", bufs=1) as xp,
            tc.tile_pool(name="accp", bufs=1) as accp,
            tc.tile_pool(name="w1p", bufs=3) as w1p,
            tc.tile_pool(name="w2p", bufs=3) as w2p,
            tc.tile_pool(name="bp", bufs=4) as bp,
            tc.tile_pool(name="hp", bufs=3) as hp,
            tc.tile_pool(name="sp", bufs=4) as sp,
            tc.tile_pool(name="gp", bufs=4) as gp,
            tc.tile_pool(name="shp", bufs=8) as shp,
            tc.tile_pool(name="psh", bufs=3, space="PSUM") as psh_pool,
            tc.tile_pool(name="pss", bufs=4, space="PSUM") as pss_pool,
            tc.tile_pool(name="psg", bufs=1, space="PSUM") as psg_pool,
        ):
            wg_sb = constp.tile([128, 2 * 4 * KD * 16], fp8)
            bg_sb = constp.tile([128, 4 * 16], f32)

            # x tiles: [128, (hi/lo, c, b)] fp8; chunked DMAs so consumers
            # start as chunks land. shared input (src 3) loads first.
            xt_sb = [
                xp.tile([128, 2 * KD * BL], fp8, name=f"xt{src}") for src in range(4)
            ]

            def xpair(src, which, cp, off, w):
                """[128, 2, w] AP over chunks (2cp, 2cp+1) of the hi (0) or
                lo (1) region of xt_sb[src], columns [off, off+w)."""
                base = xt_sb[src][:, which * KD * BL : (which + 1) * KD * BL]
                return base.rearrange("p (c b) -> p c b", c=KD)[
                    :, 2 * cp : 2 * cp + 2, off : off + w
                ]

            def load_xt(src):
                for which in range(2):
                    for c in range(KD):
                        nc.sync.dma_start(
                            out=xt_sb[src][
                                :,
                                (which * KD + c) * BL : (which * KD + c + 1) * BL,
                            ],
                            in_=xT[src][which][c * 128 : (c + 1) * 128, :],
                        )

            acc = [accp.tile([128, NBT * E], f32, name=f"acc{t}") for t in range(4)]

            gate_sb = [
                constp.tile([128, NBT * 16], f32, name=f"gate{s}") for s in range(4)
            ]
            written = set()  # (acc_idx, bt) already initialized

            def wgpair(which, src, cp):
                """[128, 2, 16] AP over gate-weight chunks (2cp, 2cp+1)."""
                base = wg_sb[:, which * 4 * KD * 16 : (which + 1) * 4 * KD * 16]
                return base.rearrange("p (s c e) -> p s c e", s=4, c=KD)[
                    :, src, 2 * cp : 2 * cp + 2, :
                ]

            def emit_gates(src):
                wexp = 8 if src < 3 else 16
                # whole-bank single accumulation group (lazy zero region):
                # bt-inner, chunk-pair outer so each arriving x chunk pair is
                # consumed immediately. 3 fp8 DoubleRow terms per pair:
                # hi.hi (main), lo.hi and hi.lo (compensation).
                psg = psg_pool.tile([128, NBT * 16], f32)
                nterm = 3 * (KD // 2) * NBT
                k = 0
                for cp in range(KD // 2):
                    for bt in range(NBT):
                        for xw, ww in ((0, 0), (1, 0), (0, 1)):
                            nc.tensor.matmul(
                                psg[:, bt * 16 : bt * 16 + 16],
                                lhsT=xpair(src, xw, cp, bt * 128, 128),
                                rhs=wgpair(ww, src, cp),
                                start=(k == 0),
                                stop=(k == nterm - 1),
                                perf_mode=DR,
                            )
                            k += 1
                # single fast eviction so the PSUM bank frees immediately;
                # softmax then runs off SBUF without stalling the next set
                logits = gp.tile([128, NBT * 16], f32, tag="logits")
                nc.scalar.copy(logits, psg)
                for bt in range(NBT):
                    logit = gp.tile([128, 16], f32, tag="logit")
                    # undo the 64x gate-weight scale, add the gate bias
                    nc.vector.scalar_tensor_tensor(
                        logit[:, :wexp],
                        logits[:, bt * 16 : bt * 16 + wexp],
                        1.0 / WSCALE,
                        bg_sb[:, src * 16 : src * 16 + wexp],
                        op0=mult,
                        op1=add,
                    )
                    g_ap = gate_sb[src][:, bt * 16 : bt * 16 + wexp]
                    ssum = gp.tile([128, 1], f32, tag="ssum")
                    nc.scalar.activation(g_ap, logit[:, :wexp], expf, accum_out=ssum)
                    rsum = gp.tile([128, 1], f32, tag="rsum")
                    nc.vector.reciprocal(rsum, ssum)
                    nc.vector.tensor_scalar_mul(g_ap, g_ap, rsum)

            # SBUF staging for shared experts computed before the task gates
            # exist: their [B,E] s tiles wait here until the gates are ready
            stage = {}

            def load_w1(e):
                w1_sb = w1p.tile([128, KD * H], fp8)
                nc.sync.dma_start(
                    out=w1_sb.rearrange("p (c h) -> p c h", c=KD),
                    in_=w1[e].rearrange("(c p) h -> p c h", p=128),
                )
                return w1_sb

            DEFAULT_BN = [(0, BN), (BN, BN)]

            def emit_expert(e, src, finalize, defer=False, extra_per_bt=None,
                            w1_pre=None, h_pre=None, bn_list=DEFAULT_BN):
                w1_sb = w1_pre if w1_pre is not None else load_w1(e)
                w1_r = w1_sb.rearrange("p (c h) -> p c h", c=KD)
                w2_sb = w2p.tile([128, KH * E], bf16)
                nc.sync.dma_start(
                    out=w2_sb.rearrange("p (c f) -> p c f", c=KH),
                    in_=w2[e].rearrange("(c p) f -> p c f", p=128),
                )
                b1_sb = bp.tile([128, KH], f32, tag="b1")
                nc.sync.dma_start(out=b1_sb, in_=b1[e])
                if use_b2:
                    b2_sb = bp.tile([128, E], f32, tag="b2")
                    nc.sync.dma_start(out=b2_sb, in_=b2bc[e])

                for off, W in bn_list:
                    if h_pre is not None and off == 0:
                        h_sb = h_pre
                        hW = BN
                    else:
                        hW = W
                        h_sb = hp.tile([128, KH * W], bf16, name="h_sb", tag="h_sb")
                        for hc in range(KH):
                            ps_h = psh_pool.tile([128, W], f32, name="ps_h", tag="ps_h")
                            for cp in range(KD // 2):
                                lh = w1_r[:, 2 * cp : 2 * cp + 2,
                                          hc * 128 : hc * 128 + 128]
                                nc.tensor.matmul(
                                    ps_h, lhsT=lh, rhs=xpair(src, 0, cp, off, W),
                                    start=(cp == 0), stop=False, perf_mode=DR,
                                )
                                nc.tensor.matmul(
                                    ps_h, lhsT=lh, rhs=xpair(src, 1, cp, off, W),
                                    start=False, stop=(cp == KD // 2 - 1),
                                    perf_mode=DR,
                                )
                            nc.scalar.activation(
                                h_sb[:, hc * W : (hc + 1) * W],
                                ps_h,
                                relu,
                                bias=b1_sb[:, hc : hc + 1],
                            )
                    for j in range(W // 128):
                        bt = off // 128 + j
                        ps_s = pss_pool.tile([128, E], f32, name="ps_s", tag="ps_s")
                        if use_b2:
                            nc.scalar.copy(ps_s, b2_sb)
                        for hc in range(KH):
                            nc.tensor.matmul(
                                ps_s,
                                lhsT=h_sb[:, hc * hW + j * 128 : hc * hW + j * 128 + 128],
                                rhs=w2_sb[:, hc * E : (hc + 1) * E],
                                start=(hc == 0 and not use_b2),
                                stop=(hc == KH - 1),
                            )
                        if defer:
                            s_sb = shp.tile([128, E], bf16, tag=f"st{e}")
                        else:
                            s_sb = sp.tile([128, E], bf16, tag="s_sb")
                        # layer-2 relu alternates ScalarE/VectorE by b-tile
                        # (GpSimd cannot read PSUM on TRN2); ScalarE also
                        # carries layer-1 relu+bias, VectorE the combines
                        if bt % 2 == 0:
                            nc.scalar.activation(s_sb, ps_s, relu)
                        else:
                            nc.vector.tensor_scalar_max(s_sb, ps_s, 0.0)
                        if defer:
                            stage[(e, bt)] = s_sb
                        else:
                            emit_contribs(e, bt, s_sb)
                        if extra_per_bt is not None:
                            extra_per_bt(bt)

                        # flush finished accumulator chunks to DRAM as soon as
                        # their last contribution lands
                        if j == W // 128 - 1:
                            nb = W // 128
                            b0 = off // 128
                            for t in finalize:
                                nc.sync.dma_start(
                                    out=out[t][off : off + W, :].rearrange(
                                        "(b p) f -> p b f", p=128
                                    ),
                                    in_=acc[t][
                                        :, b0 * E : (b0 + nb) * E
                                    ].rearrange("p (b f) -> p b f", b=nb),
                                )

            def contribs_of(e):
                if e < T * S:
                    t, s = divmod(e, S)
                    return [(t, s), (3, t * S + s)]
                jsh = e - T * S
                return [(t, S + jsh) for t in range(T)] + [(3, T * S + jsh)]

            def emit_contribs(e, bt, src_tile):
                for gset, col in contribs_of(e):
                    g = gate_sb[gset][:, bt * 16 + col : bt * 16 + col + 1]
                    a = acc[gset][:, bt * E : (bt + 1) * E]
                    if (gset, bt) not in written:
                        written.add((gset, bt))
                        nc.vector.tensor_scalar_mul(a, src_tile, g)
                    else:
                        nc.vector.scalar_tensor_tensor(
                            a, src_tile, g, a, op0=mult, op1=add
                        )

            # Emission = engine program order. Shared experts first: they need
            # no gates at compute time (combine deferred via SBUF staging), so
            # PE ramps while only xt3 + their weights are in flight. Task
            # gates follow, then spec experts with the deferred shared-pool
            # contributions interleaved per b-tile.
            finalize_at = {3: [0], 7: [1], 11: [2, 3]}

            # Prologue: xt3 and w1[12] chunk DMAs interleaved; e12's first-half
            # layer-1 runs pair-outer across 3 PSUM banks so PE consumes each
            # (xt3, w1) chunk pair as it lands instead of idling on the load.
            w1_12 = w1p.tile([128, KD * H], fp8, name="w1_12")
            for c in range(KD):
                nc.sync.dma_start(
                    out=xt_sb[3][:, c * BL : (c + 1) * BL],
                    in_=xT[3][0][c * 128 : (c + 1) * 128, :],
                )
                nc.sync.dma_start(
                    out=xt_sb[3][:, (KD + c) * BL : (KD + c + 1) * BL],
                    in_=xT[3][1][c * 128 : (c + 1) * 128, :],
                )
                nc.sync.dma_start(
                    out=w1_12[:, c * H : (c + 1) * H],
                    in_=w1[12][c * 128 : (c + 1) * 128, :],
                )
                if c == 1:
                    # mid-stream so it lands well before the gate(3) matmuls
                    nc.sync.dma_start(out=wg_sb[:, : 4 * KD * 16], in_=wg[0])
                if c == 2:
                    nc.sync.dma_start(out=wg_sb[:, 4 * KD * 16 :], in_=wg[1])
            b1_12 = bp.tile([128, KH], f32, tag="b1", name="b1_12")
            nc.sync.dma_start(out=b1_12, in_=b1[12])
            nc.sync.dma_start(out=bg_sb, in_=bg[:, :])

            w1_12r = w1_12.rearrange("p (c h) -> p c h", c=KD)
            h12 = hp.tile([128, KH * BN], bf16, name="h12")
            ph = [
                psh_pool.tile([128, BN], f32, name=f"ph{hc}", tag="ps_h")
                for hc in range(3)
            ]
            for cp in range(KD // 2):
                for hc in range(3):
                    lh = w1_12r[:, 2 * cp : 2 * cp + 2, hc * 128 : hc * 128 + 128]
                    nc.tensor.matmul(
                        ph[hc], lhsT=lh, rhs=xpair(3, 0, cp, 0, BN),
                        start=(cp == 0), stop=False, perf_mode=DR,
                    )
                    nc.tensor.matmul(
                        ph[hc], lhsT=lh, rhs=xpair(3, 1, cp, 0, BN),
                        start=False, stop=(cp == KD // 2 - 1), perf_mode=DR,
                    )
            for hc in range(3):
                nc.scalar.activation(
                    h12[:, hc * BN : (hc + 1) * BN], ph[hc], relu,
                    bias=b1_12[:, hc : hc + 1],
                )
            ph3 = psh_pool.tile([128, BN], f32, name="ph3", tag="ps_h")
            for cp in range(KD // 2):
                lh = w1_12r[:, 2 * cp : 2 * cp + 2, 3 * 128 : 4 * 128]
                nc.tensor.matmul(
                    ph3, lhsT=lh, rhs=xpair(3, 0, cp, 0, BN),
                    start=(cp == 0), stop=False, perf_mode=DR,
                )
                nc.tensor.matmul(
                    ph3, lhsT=lh, rhs=xpair(3, 1, cp, 0, BN),
                    start=False, stop=(cp == KD // 2 - 1), perf_mode=DR,
                )
            nc.scalar.activation(
                h12[:, 3 * BN : 4 * BN], ph3, relu, bias=b1_12[:, 3:4]
            )
            emit_gates(3)
            emit_expert(12, 3, [], defer=True, w1_pre=w1_12, h_pre=h12)
            load_xt(0)
            emit_gates(0)
            emit_expert(13, 3, [], defer=True)
            emit_expert(14, 3, [], defer=True)
            load_xt(1)
            emit_gates(1)
            emit_expert(15, 3, [], defer=True)
            load_xt(2)
            emit_gates(2)

            def make_hook(shared_e):
                def hook(bt):
                    emit_contribs(shared_e, bt, stage[(shared_e, bt)])
                return hook

            for e in [0, 1, 2, 3]:
                emit_expert(e, 0, finalize_at.get(e, []),
                            extra_per_bt=make_hook(12 + e))
            for e in [4, 5, 6, 7]:
                emit_expert(e, 1, finalize_at.get(e, []))
            for e in [8, 9, 10]:
                emit_expert(e, 2, finalize_at.get(e, []))
            # last expert runs progressively finer column chunks so the final
            # combine + accumulator flush pipeline covers only 128 rows
            emit_expert(11, 2, finalize_at[11],
                        bn_list=[(0, 512), (512, 256), (768, 128), (896, 128)])

    _check_read_before_write(nc)
    if split_waits:
        _split_excess_waits(nc)
    return nc


def _prep_shared(W_spec1, b_spec1, W_spec2, b_spec2, W_sh1, b_sh1, W_sh2, b_sh2,
                 W_gate, b_gate, W_gate_sh, b_gate_sh, use_b2):
    """Host-side prep of the replicated (per-core-identical) tensors."""
    w1f = np.concatenate([W_spec1, W_sh1], axis=0).astype(np.float32) * WSCALE
    w1 = np.ascontiguousarray(w1f.astype(E4M3))
    # W2 pre-divided by the 64x layer-1 scale so s comes out unscaled
    w2f = np.concatenate([W_spec2, W_sh2], axis=0).astype(np.float32) / WSCALE
    w2 = np.ascontiguousarray(w2f.astype(BF16))
    b1 = np.ascontiguousarray(
        (np.concatenate([b_spec1, b_sh1], axis=0).astype(np.float32) * WSCALE)
        .reshape(NEXP, KH, 128)
        .transpose(0, 2, 1)
    )
    # gate weights: [4 sets, D, 16] (task gates padded 8 -> 16 with zeros),
    # scaled 64x and split into an (hi, lo) e4m3 pair
    wg_full = np.zeros((4, D, 16), np.float32)
    wg_full[:T, :, : S + NS] = W_gate
    wg_full[3] = W_gate_sh
    wg_full *= WSCALE
    # device layout [128, (set, chunk, e)]
    wg_lay = (
        wg_full.reshape(4, KD, 128, 16).transpose(2, 0, 1, 3).reshape(128, 4 * KD * 16)
    )
    wg_hi = wg_lay.astype(E4M3)
    wg_lo = (wg_lay - wg_hi.astype(np.float32)).astype(E4M3)
    wg = np.ascontiguousarray(np.stack([wg_hi, wg_lo], axis=0))
    bg_full = np.zeros((4, 16), np.float32)
    bg_full[:T, : S + NS] = b_gate
    bg_full[3] = b_gate_sh
    bg = np.ascontiguousarray(
        np.broadcast_to(bg_full[None], (128, 4, 16)).reshape(128, 64)
    )
    shared = dict(w1=w1, w2=w2, b1=b1, wg=wg, bg=bg)
    if use_b2:
        b2f = np.concatenate([b_spec2, b_sh2], axis=0).astype(np.float32)
        shared["b2bc"] = np.ascontiguousarray(
            np.broadcast_to(b2f[:, None, :], (NEXP, 128, E))
        )
    return shared


def kernel(x_tasks, x_shared, W_spec1, b_spec1, W_spec2, b_spec2,
           W_sh1, b_sh1, W_sh2, b_sh2, W_gate, b_gate, W_gate_sh, b_gate_sh):
    global LAST_EXEC_NS
    use_b2 = bool(np.any(b_spec2)) or bool(np.any(b_sh2))
    key = ("nc", use_b2)
    if key not in _CACHE:
        _CACHE[key] = _build_program(use_b2=use_b2)
    nc = _CACHE[key]
    _CACHE["nc"] = nc  # for test harness TimelineSim

    shared = _prep_shared(W_spec1, b_spec1, W_spec2, b_spec2, W_sh1, b_sh1,
                          W_sh2, b_sh2, W_gate, b_gate, W_gate_sh, b_gate_sh,
                          use_b2)

    x_tasks = np.asarray(x_tasks, np.float32)
    x_shared = np.asarray(x_shared, np.float32)

    # feature-major x, split into an e4m3 (hi, lo) pair (hi + lo ~ bf16-level
    # precision once both products are accumulated)
    xall = np.empty((4, D, B), np.float32)
    for t in range(T):
        xall[t] = x_tasks[t].T
    xall[3] = x_shared.T
    xhi = xall.astype(E4M3)
    xlo = (xall - xhi.astype(np.float32)).astype(E4M3)

    in_maps = []
    for i in range(NCORES):
        sl = slice(i * BL, (i + 1) * BL)
        xt = np.empty((4, 2, D, BL), E4M3)
        xt[:, 0] = xhi[:, :, sl]
        xt[:, 1] = xlo[:, :, sl]
        m = {"xT": np.ascontiguousarray(xt)}
        m.update(shared)
        in_maps.append(m)

    res = run_bass_kernel_spmd(nc, in_maps, core_ids=list(range(NCORES)), trace=TRACE)
    LAST_EXEC_NS = res.exec_time_ns

    full = np.empty((T + 1, B, E), np.float32)
    for i in range(NCORES):
        full[:, i * BL : (i + 1) * BL, :] = res.results[i]["out"]
    return full
